# revision 6
# baseline (speedup 1.0000x reference)
"""CapsuleLayer forward (squash + per-capsule matmul) on 8 Trainium2 cores.

Reference computation (all fp32):
    x  = inputs.reshape(B, 1152, 8)
    pc = squash(x)                              # per-(b,n) over k=8
    u_hat[b,n,j,d] = sum_k W[0,n,j,d,k] * pc[b,n,k]
    out = u_hat[..., None]                      # [B, 1152, 10, 16, 1]

Sharding: capsule dim (n=1152) split 144-per-core across 8 cores; every core
keeps the full batch (B=512).  Zero cross-device communication.

Per-core kernel (fp16 data paths; PSUM accumulates fp32):
  - squash scale simplified algebraically: sq/((1+sq)*sqrt(sq+eps))
    == sqrt(sq)/(1+sq) (eps negligible, NaN-safe at sq=0); short DVE chain
    plus ACT sqrt/add, split 16-caps-first on chunk 0 so the PE starts
    within ~1us of the x arrival
  - W packed host-side as K=64 block-diagonal: per 16-cap group two
    8-cap-block [64, 1280] fp16 tiles (base partition 0 each, so every
    matmul is a standard K=64/base-0 op).  8x zero-padding -> 2.95MB W
    traffic (vs 5.9MB for the flat K=128 block-diagonal), no on-chip
    expansion, same total PE moving columns (2560/group)
  - all 4 x-chunk DMAs issued up-front on the scalar ring; W tiles stream
    on the sync ring in group order, so nothing head-of-line blocks
  - pc transposed to [ck, b] via PE transpose, pipelined one group ahead
  - per group 8 matmuls (pieces cut at the 512 PSUM-bank and 640 block
    boundaries), PSUM->SBUF evacuation pa->DVE, pb+pcs->ACT; output
    stored per group in [128, 2560] fp16 tiles alternating the
    sync/scalar HWDGE rings so SDMA engines round-robin two queue rows
    and hide per-DMA completion-receipt bubbles
"""

from contextlib import ExitStack

import numpy as np

import concourse.bacc as bacc
import concourse.bass as bass  # noqa: F401  (AP helpers)
import concourse.mybir as mybir
import concourse.tile as tile
from concourse.bass_utils import run_bass_kernel_spmd
from concourse.masks import make_identity

N_CORES = 8
B = 512
N_CAPS = 1152
K = 8
JD = 160  # 10*16
CAPS_PER_CORE = N_CAPS // N_CORES  # 144
GROUP_CAPS = 16  # caps per output group -> 2560 cols
N_GROUPS = CAPS_PER_CORE // GROUP_CAPS  # 9
GROUP_COLS = GROUP_CAPS * JD  # 2560
BLK_CAPS = 8  # caps per matmul block -> K=64 (base partitions 0/64 only)
BLK_COLS = BLK_CAPS * JD  # 1280
BLK_K = BLK_CAPS * K  # 64
N_BLKS = GROUP_CAPS // BLK_CAPS  # 2 blocks stacked per [128, 1280] tile
P = 128
B_CHUNKS = B // P  # 4

# matmul pieces: cut at 512 (PSUM bank) and 640 (block) boundaries
_cuts = sorted(
    {0, GROUP_COLS}
    | {c for c in range(512, GROUP_COLS, 512)}
    | {c for c in range(BLK_COLS, GROUP_COLS, BLK_COLS)}
)
PIECES = list(zip(_cuts[:-1], _cuts[1:]))  # [(0,512),(512,640),...]

F32 = mybir.dt.float32
F16 = mybir.dt.float16
OUT_DT = mybir.dt.float16
OUT_NP = np.float16


def build_program():
    nc = bacc.Bacc("TRN2", debug=False, num_devices=N_CORES)
    x = nc.dram_tensor("x", [B, CAPS_PER_CORE * K], F16, kind="ExternalInput").ap()
    wb = nc.dram_tensor(
        "wb", [N_GROUPS * P, BLK_COLS], F16, kind="ExternalInput"
    ).ap()
    out = nc.dram_tensor(
        "out", [B, CAPS_PER_CORE * JD], OUT_DT, kind="ExternalOutput"
    ).ap()

    with tile.TileContext(nc) as tc, ExitStack() as ctx:
        consts = ctx.enter_context(tc.tile_pool(name="consts", bufs=1))
        wblk_pool = ctx.enter_context(tc.tile_pool(name="wblk", bufs=1))
        xpool = ctx.enter_context(tc.tile_pool(name="xpool", bufs=4))
        x2pool = ctx.enter_context(tc.tile_pool(name="x2pool", bufs=2))
        pcpool = ctx.enter_context(tc.tile_pool(name="pcpool", bufs=2))
        stats = ctx.enter_context(tc.tile_pool(name="stats", bufs=2))
        pct_pool = ctx.enter_context(tc.tile_pool(name="pct", bufs=3))
        ost_pool = ctx.enter_context(tc.tile_pool(name="ost", bufs=4))
        # PSUM: 3x 2-bank matmul slots + 2x 1-bank transpose slots = 8 banks.
        psum = ctx.enter_context(tc.tile_pool(name="psum", bufs=3, space="PSUM"))
        psum_t = ctx.enter_context(tc.tile_pool(name="psum_t", bufs=2, space="PSUM"))

        # x chunk 0 first on the scalar ring (its squash chain is the
        # critical path); W group tiles stream on the sync ring in the
        # order the PE consumes them.
        xts = []
        xt0 = xpool.tile([P, CAPS_PER_CORE, K], F16, tag="xt0")
        nc.scalar.dma_start(
            out=xt0, in_=x[0:P, :].rearrange("b (c k) -> b c k", k=K)
        )
        xts.append(xt0)
        wblk = []
        for g in range(N_GROUPS):
            blks = []
            for j in range(N_BLKS):
                wt = wblk_pool.tile(
                    [BLK_K, BLK_COLS], F16, tag=f"wblk{g}_{j}", name=f"wblk{g}_{j}"
                )
                r0 = g * P + j * BLK_K
                nc.sync.dma_start(out=wt, in_=wb[r0 : r0 + BLK_K, :])
                blks.append(wt)
            wblk.append(blks)
        for bi in range(1, B_CHUNKS):
            xt = xpool.tile([P, CAPS_PER_CORE, K], F16, tag=f"xt{bi}")
            nc.scalar.dma_start(
                out=xt,
                in_=x[bi * P : (bi + 1) * P, :].rearrange("b (c k) -> b c k", k=K),
            )
            xts.append(xt)

        identity = consts.tile([P, P], F16)
        make_identity(nc, identity)

        def emit_chain(xt, pc, c0, c1):
            # scale[b,c] = sqrt(sq)/(1+sq), pc = x*scale over caps [c0,c1)
            ncap = c1 - c0
            xs = xt[:, c0:c1, :]
            x2 = x2pool.tile([P, ncap, K], F16, tag=f"x2_{ncap}")
            nc.vector.tensor_mul(x2, xs, xs)
            sq = stats.tile([P, ncap], F16, tag=f"sq_{ncap}")
            nc.vector.reduce_sum(out=sq, in_=x2, axis=mybir.AxisListType.X)
            sn = stats.tile([P, ncap], F16, tag=f"sn_{ncap}")
            nc.scalar.activation(
                out=sn, in_=sq, func=mybir.ActivationFunctionType.Sqrt
            )
            t1 = stats.tile([P, ncap], F16, tag=f"t1_{ncap}")
            nc.scalar.activation(
                out=t1, in_=sq, func=mybir.ActivationFunctionType.Identity,
                bias=1.0,
            )
            rden = stats.tile([P, ncap], F16, tag=f"rd_{ncap}")
            nc.vector.reciprocal(rden, t1)
            scale = stats.tile([P, ncap], F16, tag=f"sc_{ncap}")
            nc.vector.tensor_mul(scale, sn, rden)
            nc.vector.tensor_mul(
                pc[:, c0:c1, :],
                xs,
                scale.unsqueeze(2).broadcast_to([P, ncap, K]),
            )

        def issue_transpose(pc_flat, g):
            # Pipelined one group ahead so the PE never waits on the
            # PSUM->SBUF pcT copies.  The two 8-cap halves land in
            # separate base-0 [64, 128] tiles (matmul operands must share
            # base partition with the [64, 1280] W block tiles).
            pst = psum_t.tile([P, P], F16, tag="pt")
            nc.tensor.transpose(pst, pc_flat[:, g * P : (g + 1) * P], identity)
            pcTs = []
            for j in range(N_BLKS):
                pcT = pct_pool.tile([BLK_K, P], F16, tag=f"pcT{j}", name=f"pcT{j}")
                src_half = pst[j * BLK_K : (j + 1) * BLK_K, :]
                if (g + j) % 2 == 0:
                    nc.scalar.copy(pcT, src_half)
                else:
                    nc.vector.tensor_copy(pcT, src_half)
                pcTs.append(pcT)
            return pcTs

        with nc.allow_low_precision("fp16 squash: tolerance is 2e-2"):
            pc0 = pcpool.tile([P, CAPS_PER_CORE, K], F16, tag="pc")
            emit_chain(xts[0], pc0, 0, GROUP_CAPS)
            emit_chain(xts[0], pc0, GROUP_CAPS, CAPS_PER_CORE)

            pc_cur = pc0.rearrange("p c k -> p (c k)")
            pc_next = None
            for bi in range(B_CHUNKS):
                pcT_next = issue_transpose(pc_cur, 0)
                for g in range(N_GROUPS):
                    pcT = pcT_next
                    if g + 1 < N_GROUPS:
                        pcT_next = issue_transpose(pc_cur, g + 1)

                    if g == 4 and bi + 1 < B_CHUNKS:
                        # Next chunk's scale chain: inputs already resident,
                        # so these DVE/ACT ops never stall their queues.
                        pcn = pcpool.tile([P, CAPS_PER_CORE, K], F16, tag="pc")
                        emit_chain(xts[bi + 1], pcn, 0, CAPS_PER_CORE)
                        pc_next = pcn.rearrange("p c k -> p (c k)")

                    pa = psum.tile([P, 1024], F32, tag="pm")
                    pb = psum.tile([P, 1024], F32, tag="pm")
                    pcs = psum.tile([P, 512], F32, tag="pm")
                    psums = {0: pa, 1024: pb, 2048: pcs}
                    for lo, hi in PIECES:
                        j = lo // BLK_COLS
                        pbase = 0 if lo < 1024 else (1024 if lo < 2048 else 2048)
                        nc.tensor.matmul(
                            psums[pbase][:, lo - pbase : hi - pbase],
                            lhsT=pcT[j],
                            rhs=wblk[g][j][
                                :, lo - j * BLK_COLS : hi - j * BLK_COLS
                            ],
                            start=True,
                            stop=True,
                        )

                    ost = ost_pool.tile([P, GROUP_COLS], OUT_DT)
                    # pa evacuates first on DVE (the next group's first
                    # matmul reuses its bank pair); ACT takes pb + pcs.
                    nc.vector.tensor_copy(ost[:, 0:1024], pa)
                    nc.scalar.copy(ost[:, 1024:2048], pb)
                    nc.scalar.copy(ost[:, 2048:2560], pcs)
                    # Alternate the two HWDGE rings per store.
                    st_eng = nc.sync if (bi * N_GROUPS + g) % 2 == 0 else nc.scalar
                    st_eng.dma_start(
                        out=out[
                            bi * P : (bi + 1) * P,
                            g * GROUP_COLS : (g + 1) * GROUP_COLS,
                        ],
                        in_=ost,
                    )
                pc_cur = pc_next
    nc.compile()
    return nc


_PROGRAM = None


def _get_program():
    global _PROGRAM
    if _PROGRAM is None:
        _PROGRAM = build_program()
    return _PROGRAM


def shard_inputs(inputs: np.ndarray, W: np.ndarray) -> list[dict[str, np.ndarray]]:
    # K=64 stacked block-diagonal W per core: per 16-cap group one
    # [128, 1280] tile; rows (c,k) = c*8+k; block j = c//8 lives in rows
    # 64j..64j+64 with W[n=c] at cols (c%8)*160..(c%8+1)*160, zeros elsewhere.
    w0 = np.asarray(W[0], dtype=np.float32).reshape(N_CAPS, JD, K)
    x16 = np.asarray(inputs, dtype=np.float16)
    in_maps = []
    for i in range(N_CORES):
        c0 = i * CAPS_PER_CORE
        wcore = w0[c0 : c0 + CAPS_PER_CORE]  # [144, 160, 8]
        wb4 = np.zeros((N_GROUPS, GROUP_CAPS, K, BLK_CAPS, JD), dtype=np.float16)
        for c in range(GROUP_CAPS):
            # wb4[g, c, k, c%4, jd] = W[g*16+c, jd, k]
            wb4[:, c, :, c % BLK_CAPS, :] = (
                wcore.reshape(N_GROUPS, GROUP_CAPS, JD, K)[:, c]
                .transpose(0, 2, 1)
            )
        in_maps.append(
            {
                "x": np.ascontiguousarray(
                    x16[:, c0 * K : (c0 + CAPS_PER_CORE) * K]
                ),
                "wb": wb4.reshape(N_GROUPS * P, BLK_COLS),
            }
        )
    return in_maps


def unshard_output(results: list[dict[str, np.ndarray]]) -> np.ndarray:
    full = np.empty((B, N_CAPS, JD), dtype=np.float32)
    for i in range(N_CORES):
        c0 = i * CAPS_PER_CORE
        full[:, c0 : c0 + CAPS_PER_CORE, :] = results[i]["out"].reshape(
            B, CAPS_PER_CORE, JD
        ).astype(np.float32)
    return full.reshape(B, N_CAPS, 10, 16, 1)


def kernel(inputs: np.ndarray, W: np.ndarray) -> np.ndarray:
    nc = _get_program()
    in_maps = shard_inputs(np.asarray(inputs), np.asarray(W))
    res = run_bass_kernel_spmd(nc, in_maps, core_ids=list(range(N_CORES)))
    return unshard_output(res.results)
